# revision 1
# baseline (speedup 1.0000x reference)
"""Contrastive segment-reduce loss kernel for Trainium2 (8 NeuronCores).

Math (equivalent to the reference):
  counts[l] = #voxels with label l                       (host bincount, exact)
  sums[l,c]  = sum_{v: id_v=l} p[v,c]                    (device matmul)
  usums[l,c] = sum_{v: id_v=l} p[v,c]/||p_v||            (device matmul)
  means = sums / max(counts,1)
  intra_sum[l] = usums[l] . means[l] / ||means[l]||      (== sum of per-voxel cos)
  intra = mean over l=1..50 of intra_sum[l]/max(counts[l],1)
  inter = mean of clip(upper-tri cosine of means[1:],0,1)
  loss = inter - intra
The per-voxel eps clamp max(pn*mn, eps) never binds for this data
(pn ~ chi(16) >= O(1), mn ~ 1e-2), so the factored form is exact.

Device strategy per core (1/8 of the voxels, data-parallel over (b, z*y*x)):
  - host ships feats[v, 0:16] = p, feats[v, 16:32] = p/||p|| as fp8e4m3
    (upcast to bf16 during the SWDGE DMA) and ids as uint8 (upcast to bf16),
    in an SBUF-friendly [T, 128, 32|1, G] layout
  - one-hot built on DVE: 51x tensor_scalar(is_equal, label) over [128, G] tiles
  - segment sums via TensorE: for each 128-voxel chunk,
    psum[32,51] += feats_chunk[128,32].T @ onehot_chunk[128,51]
  - single [32,51] fp32 result per core, reduced on host.
"""

import numpy as np
import ml_dtypes

import concourse.tile as tile
from concourse import bacc, mybir
from concourse.bass_utils import run_bass_kernel_spmd

NUM_LABELS = 51
EPS = 1e-8

N_CORES = 8
B, C, Z, Y, X = 2, 16, 32, 256, 256
NV_TOTAL = B * Z * Y * X            # 4_194_304 voxels
NV_CORE = NV_TOTAL // N_CORES       # 524_288 voxels per core
P = 128                             # partitions
G = 512                             # voxels per partition per tile
T = NV_CORE // (P * G)              # 8 tiles per core
F = 32                              # feature columns: 16 raw p + 16 unit p

_cache = {}


def _build_bass(t_tiles=T, g=G, pack=1):
    L = NUM_LABELS
    nc = bacc.Bacc(
        "TRN2",
        target_bir_lowering=False,
        debug=False,
        enable_asserts=False,
        num_devices=N_CORES,
    )
    p_d = nc.dram_tensor(
        "p", [t_tiles, P, (F // 2) * g], mybir.dt.bfloat16, kind="ExternalInput"
    )
    u_d = nc.dram_tensor(
        "u", [t_tiles, P, (F // 2) * g], mybir.dt.float8e4, kind="ExternalInput"
    )
    ids_d = nc.dram_tensor("ids", [t_tiles, P, g], mybir.dt.uint8, kind="ExternalInput")
    out_d = nc.dram_tensor(
        "out", [pack * F, L], mybir.dt.float32, kind="ExternalOutput"
    )

    with tile.TileContext(nc) as tc:
        with (
            tc.tile_pool(name="fpool", bufs=2) as fpool,
            tc.tile_pool(name="ipool", bufs=1) as ipool,
            tc.tile_pool(name="ohpool", bufs=2) as ohpool,
            tc.tile_pool(name="opool", bufs=1) as opool,
            tc.tile_pool(name="psum", bufs=1, space="PSUM") as psum_pool,
        ):
            acc = psum_pool.tile([pack * F, L], dtype=mybir.dt.float32, space="PSUM")
            # all ids upfront in one DMA (uint8 -> bf16 cast in the DMA)
            ids_sb = ipool.tile([P, t_tiles * g], mybir.dt.bfloat16)
            nc.gpsimd.dma_start(
                out=ids_sb[:].rearrange("p (t g) -> p t g", g=g),
                in_=ids_d.ap()[:, :, :].rearrange("t p g -> p t g"),
            )
            for t in range(t_tiles):
                ftile = fpool.tile([P, F * g], mybir.dt.bfloat16)
                # p: straight bf16 on HWDGE; u: fp8->bf16 cast on SWDGE.
                # Disjoint halves of one tile so the per-chunk stationary AP
                # [p | u] stays a single strided view.
                nc.sync.dma_start(out=ftile[:, : (F // 2) * g], in_=p_d.ap()[t])
                nc.gpsimd.dma_start(out=ftile[:, (F // 2) * g :], in_=u_d.ap()[t])

                oh = ohpool.tile([P, L * g], mybir.dt.bfloat16)
                for l in range(L):
                    nc.vector.tensor_scalar(
                        out=oh[:, l * g : (l + 1) * g],
                        in0=ids_sb[:, t * g : (t + 1) * g],
                        scalar1=float(l),
                        scalar2=None,
                        op0=mybir.AluOpType.is_equal,
                    )

                # [P, F, G] / [P, L, G] views; chunk g uses column g of each.
                f_r = ftile[:].rearrange("p (f g) -> p g f", g=g)
                oh_r = oh[:].rearrange("p (l g) -> p g l", g=g)
                if pack == 1:
                    for gi in range(g):
                        nc.tensor.matmul(
                            out=acc[:],
                            lhsT=f_r[:, gi, :],
                            rhs=oh_r[:, gi, :],
                            start=(t == 0 and gi == 0),
                            stop=(t == t_tiles - 1 and gi == g - 1),
                        )
                else:
                    # pack chunks into disjoint PE column groups; MMs to
                    # different col groups run concurrently in the array
                    for q in range(g // pack):
                        for gg in range(pack):
                            gi = q * pack + gg
                            nc.tensor.matmul(
                                out=acc[F * gg : F * (gg + 1), :],
                                lhsT=f_r[:, gi, :],
                                rhs=oh_r[:, gi, :],
                                start=(t == 0 and q == 0),
                                stop=(t == t_tiles - 1 and q == g // pack - 1),
                                tile_position=(0, F * gg),
                                skip_group_check=True,
                            )

            res = opool.tile([pack * F, L], mybir.dt.float32)
            nc.vector.tensor_copy(out=res[:], in_=acc[:])
            nc.sync.dma_start(out=out_d.ap()[:, :], in_=res[:])
    nc.compile()
    return nc


def _host_prep(prediction, gt):
    """Build per-core device inputs. Returns (in_maps, counts)."""
    pred = np.asarray(prediction, dtype=np.float32)
    ids64 = np.asarray(gt)
    counts = np.bincount(ids64.reshape(-1).astype(np.int64), minlength=NUM_LABELS)

    predf = pred.reshape(B, C, -1)
    nrm = np.sqrt(np.einsum("bcv,bcv->bv", predf, predf))
    u = predf / np.maximum(nrm, 1e-30)[:, None, :]

    nvb = predf.shape[2]
    per_core = nvb // (N_CORES // B)
    half = F // 2
    in_maps = []
    for k in range(N_CORES):
        b, q = divmod(k, N_CORES // B)
        csl = slice(q * per_core, (q + 1) * per_core)
        p_t = np.ascontiguousarray(
            predf[b, :, csl].reshape(half, T, P, G).transpose(1, 2, 0, 3)
            .astype(ml_dtypes.bfloat16)
        ).reshape(T, P, half * G)
        u_t = np.ascontiguousarray(
            u[b, :, csl].reshape(half, T, P, G).transpose(1, 2, 0, 3)
            .astype(ml_dtypes.float8_e4m3fn)
        ).reshape(T, P, half * G)
        ids_sl = ids64.reshape(B, -1)[b, csl]
        idt = np.ascontiguousarray(ids_sl.reshape(T, P, G).astype(np.uint8))
        in_maps.append({"p": p_t, "u": u_t, "ids": idt})
    return in_maps, counts


def _host_final(outs, counts):
    """outs: list of [F, 51] fp32 per core. Final tiny reduction in float64."""
    tot = np.zeros((F, NUM_LABELS), dtype=np.float64)
    for o in outs:
        tot += o.astype(np.float64).reshape(-1, F, NUM_LABELS).sum(axis=0)
    sums = tot[0:16, :].T       # [51, 16]
    usums = tot[16:32, :].T     # [51, 16]
    cnt = counts.astype(np.float64)

    means = sums / np.maximum(cnt, 1.0)[:, None]
    mn = np.linalg.norm(means, axis=1)
    intra_sum = np.einsum("lc,lc->l", usums, means) / np.maximum(mn, 1e-300)
    intra_per_label = intra_sum[1:] / np.maximum(cnt[1:], 1.0)
    intra = intra_per_label.mean()

    cm = means[1:]
    cmn = cm / np.maximum(np.linalg.norm(cm, axis=1, keepdims=True), EPS)
    gram = cmn @ cmn.T
    iu, ju = np.triu_indices(NUM_LABELS - 1, k=1)
    inter = np.clip(gram[iu, ju], 0.0, 1.0).mean()
    return np.float32(inter - intra)


def kernel(prediction, gt):
    in_maps, counts = _host_prep(prediction, gt)
    if "nc" not in _cache:
        _cache["nc"] = _build_bass()
    res = run_bass_kernel_spmd(_cache["nc"], in_maps, core_ids=list(range(N_CORES)))
    outs = [r["out"] for r in res.results]
    return _host_final(outs, counts)


if __name__ == "__main__":
    rng = np.random.default_rng(0)
    pred = rng.standard_normal((B, C, Z, Y, X), dtype=np.float32)
    gt = rng.integers(0, NUM_LABELS, size=(B, Z, Y, X)).astype(np.int64)
    print("loss:", kernel(pred, gt))



# revision 3
# speedup vs baseline: 3.0089x; 3.0089x over previous
"""Contrastive segment-reduce loss kernel for Trainium2 (8 NeuronCores).

Math (equivalent to the reference):
  counts[l] = #voxels with label l                     (host bincount, exact)
  sums[l,c]  = sum_{v: id_v=l} p[v,c]   = sum n_v * u_v[c]
  usums[l,c] = sum_{v: id_v=l} u_v[c],  u_v = p_v/||p_v||, n_v = ||p_v||
  means = sums / max(counts,1)
  intra_sum[l] = usums[l] . means[l] / ||means[l]||    (== sum of per-voxel cos)
  intra = mean over l=1..50 of intra_sum[l]/max(counts[l],1)
  inter = mean of clip(upper-tri cosine of means[1:],0,1)
  loss = inter - intra
The per-voxel eps clamp max(pn*mn, eps) never binds for this data
(pn ~ chi(16) >= O(1), mn ~ 1e-2), so the factored form is exact.

Device strategy (sort-based, no one-hot, no ids on device):
  - host sorts each batch's voxels by label, ships u = p/||p|| as fp8e4m3
    [128, G, 16] and a moving tensor m[128, G, 2] = [1 | n] bf16
  - device computes PER-CHUNK sums via TensorE only: for 128-voxel chunk g,
      psum[0:16, 2g:2g+2] = u_chunk[128,16].T @ m_chunk[128,2]
    i.e. column 2g = sum of u over the chunk (usums), column 2g+1 = sum of
    n*u = sum of p (sums). 4096 chunks/core, ap_size=2 -> PE nearly free.
  - chunk -> label mapping is known on host (sorted order); label sums are
    prefix-sum differences of chunk sums plus exact host-side corrections
    for the <=50 chunks per batch that straddle a label boundary.
  - per-core DMA: 8 MiB u + 2 MiB m in, 512 KiB chunk sums out
    (vs 24.5 MiB for the one-hot/matmul formulation) -> memory-roofline.
"""

import numpy as np
import ml_dtypes

import concourse.tile as tile
from concourse import bacc, mybir
from concourse.bass_utils import run_bass_kernel_spmd

NUM_LABELS = 51
EPS = 1e-8

N_CORES = 8
B, C, Z, Y, X = 2, 16, 32, 256, 256
NVB = Z * Y * X                     # voxels per batch = 2_097_152
CORES_PER_B = N_CORES // B          # 4
NV_CORE = NVB // CORES_PER_B        # 524_288 voxels per core
P = 128                             # partitions = voxels per chunk
CH = 16                             # channels
G = 1024                            # chunks per tile (one PSUM flush)
T = NV_CORE // (P * G)              # 4 tiles per core
CHUNKS_CORE = T * G                 # 4096
CHUNKS_B = CHUNKS_CORE * CORES_PER_B  # 16384 chunks per batch

_cache = {}


def _build_bass():
    nc = bacc.Bacc(
        "TRN2",
        target_bir_lowering=False,
        debug=False,
        enable_asserts=False,
        num_devices=N_CORES,
    )
    u_d = nc.dram_tensor("u", [T, P, G * CH], mybir.dt.float8e4, kind="ExternalInput")
    m_d = nc.dram_tensor("m", [T, P, G * 2], mybir.dt.bfloat16, kind="ExternalInput")
    out_d = nc.dram_tensor("out", [T, CH, G * 2], mybir.dt.float32, kind="ExternalOutput")

    with tile.TileContext(nc) as tc:
        with (
            tc.tile_pool(name="upool", bufs=2) as upool,
            tc.tile_pool(name="mpool", bufs=2) as mpool,
            tc.tile_pool(name="rpool", bufs=2) as rpool,
            tc.tile_pool(name="psum", bufs=2, space="PSUM") as psum_pool,
        ):
            for t in range(T):
                ut = upool.tile([P, G * CH], mybir.dt.float8e4)
                mt = mpool.tile([P, G * 2], mybir.dt.bfloat16)
                nc.sync.dma_start(out=ut[:], in_=u_d.ap()[t])
                nc.scalar.dma_start(out=mt[:], in_=m_d.ap()[t])

                acc = psum_pool.tile([CH, G * 2], dtype=mybir.dt.float32, space="PSUM")
                for g in range(G):
                    nc.tensor.matmul(
                        out=acc[:, 2 * g : 2 * g + 2],
                        lhsT=ut[:, CH * g : CH * (g + 1)],
                        rhs=mt[:, 2 * g : 2 * g + 2],
                        start=True,
                        stop=True,
                    )
                res = rpool.tile([CH, G * 2], mybir.dt.float32)
                nc.vector.tensor_copy(out=res[:], in_=acc[:])
                nc.sync.dma_start(out=out_d.ap()[t], in_=res[:])
    nc.compile()
    return nc


def _host_prep(prediction, gt):
    """Sort voxels by label per batch; build per-core device inputs.

    Returns (in_maps, counts, per_batch) where per_batch[b] =
    (u8s [NVB,16] fp8, n16s [NVB] bf16, starts [52] int64) in sorted order.
    """
    pred = np.asarray(prediction, dtype=np.float32)
    ids64 = np.asarray(gt)
    counts = np.bincount(ids64.reshape(-1).astype(np.int64), minlength=NUM_LABELS)

    p = pred.reshape(B, C, -1)
    ids = ids64.reshape(B, -1).astype(np.int32)
    in_maps = [None] * N_CORES
    per_batch = []
    for b in range(B):
        nrm = np.sqrt(np.einsum("cv,cv->v", p[b], p[b]))
        u8 = (p[b] / np.maximum(nrm, 1e-30)[None, :]).astype(ml_dtypes.float8_e4m3fn)
        n16 = nrm.astype(ml_dtypes.bfloat16)
        counts_b = np.bincount(ids[b], minlength=NUM_LABELS)
        starts = np.zeros(NUM_LABELS + 1, np.int64)
        starts[1:] = np.cumsum(counts_b)
        order = np.argsort(ids[b], kind="stable")
        u8s = np.ascontiguousarray(u8[:, order].T)      # [NVB, 16] fp8
        n16s = np.ascontiguousarray(n16[order])         # [NVB] bf16
        per_batch.append((u8s, n16s, starts))
        for q in range(CORES_PER_B):
            sl = slice(q * NV_CORE, (q + 1) * NV_CORE)
            us = np.ascontiguousarray(
                u8s[sl].reshape(T, G, P, CH).transpose(0, 2, 1, 3)
            ).reshape(T, P, G * CH)
            m = np.empty((T, P, G, 2), ml_dtypes.bfloat16)
            m[..., 0] = np.asarray(1.0, ml_dtypes.bfloat16)
            m[..., 1] = n16s[sl].reshape(T, G, P).transpose(0, 2, 1)
            in_maps[b * CORES_PER_B + q] = {"u": us, "m": m.reshape(T, P, G * 2)}
    return in_maps, counts, per_batch


def _host_final(outs, counts, per_batch):
    """outs: per core [T, CH, G*2] fp32 chunk sums. Final reduce in float64."""
    sums = np.zeros((NUM_LABELS, CH), np.float64)
    usums = np.zeros((NUM_LABELS, CH), np.float64)
    for b in range(B):
        u8s, n16s, starts = per_batch[b]
        cs = np.concatenate(
            [
                np.asarray(outs[b * CORES_PER_B + q], np.float64)
                .reshape(T, CH, G, 2)
                .transpose(0, 2, 3, 1)
                .reshape(T * G, 2, CH)
                for q in range(CORES_PER_B)
            ]
        )  # [CHUNKS_B, 2, CH]: [:,0]=usum chunk, [:,1]=psum chunk
        pref = np.zeros((CHUNKS_B + 1, 2, CH), np.float64)
        np.cumsum(cs, axis=0, out=pref[1:])
        for l in range(NUM_LABELS):
            s, e = int(starts[l]), int(starts[l + 1])
            if s == e:
                continue
            lo, hi = -(-s // P), e // P
            if hi > lo:
                usums[l] += pref[hi, 0] - pref[lo, 0]
                sums[l] += pref[hi, 1] - pref[lo, 1]
                head = (s, lo * P)
                tailr = (hi * P, e)
            else:
                head = (s, e)
                tailr = (0, 0)
            for a, z in (head, tailr):
                if z > a:
                    useg = u8s[a:z].astype(np.float64)
                    nseg = n16s[a:z].astype(np.float64)
                    usums[l] += useg.sum(axis=0)
                    sums[l] += (useg * nseg[:, None]).sum(axis=0)

    cnt = counts.astype(np.float64)
    means = sums / np.maximum(cnt, 1.0)[:, None]
    mn = np.linalg.norm(means, axis=1)
    intra_sum = np.einsum("lc,lc->l", usums, means) / np.maximum(mn, 1e-300)
    intra_per_label = intra_sum[1:] / np.maximum(cnt[1:], 1.0)
    intra = intra_per_label.mean()

    cm = means[1:]
    cmn = cm / np.maximum(np.linalg.norm(cm, axis=1, keepdims=True), EPS)
    gram = cmn @ cmn.T
    iu, ju = np.triu_indices(NUM_LABELS - 1, k=1)
    inter = np.clip(gram[iu, ju], 0.0, 1.0).mean()
    return np.float32(inter - intra)


def kernel(prediction, gt):
    in_maps, counts, per_batch = _host_prep(prediction, gt)
    if "nc" not in _cache:
        _cache["nc"] = _build_bass()
    res = run_bass_kernel_spmd(_cache["nc"], in_maps, core_ids=list(range(N_CORES)))
    outs = [r["out"] for r in res.results]
    return _host_final(outs, counts, per_batch)


if __name__ == "__main__":
    rng = np.random.default_rng(0)
    pred = rng.standard_normal((B, C, Z, Y, X), dtype=np.float32)
    gt = rng.integers(0, NUM_LABELS, size=(B, Z, Y, X)).astype(np.int64)
    print("loss:", kernel(pred, gt))


# revision 12
# speedup vs baseline: 3.2950x; 1.0951x over previous
"""Contrastive segment-reduce loss kernel for Trainium2 (8 NeuronCores).

Math (equivalent to the reference):
  counts[l] = #voxels with label l                     (host bincount, exact)
  sums[l,c]  = sum_{v: id_v=l} p[v,c]   = sum n_v * u_v[c]
  usums[l,c] = sum_{v: id_v=l} u_v[c],  u_v = p_v/||p_v||, n_v = ||p_v||
  means = sums / max(counts,1)
  intra_sum[l] = usums[l] . means[l] / ||means[l]||    (== sum of per-voxel cos)
  intra = mean over l=1..50 of intra_sum[l]/max(counts[l],1)
  inter = mean of clip(upper-tri cosine of means[1:],0,1)
  loss = inter - intra
The per-voxel eps clamp max(pn*mn, eps) never binds for this data
(pn ~ chi(16) >= O(1), mn ~ 1e-2), so the factored form is exact.

Device strategy (sort-based, no one-hot, no ids on device):
  - host sorts each batch's voxels by label, ships u = p/||p|| as fp8e4m3
    [128, G, 16] and a moving tensor m[128, G, 2] = [1 | n] bf16
  - device computes PER-CHUNK sums via TensorE only: for 128-voxel chunk g,
      psum[0:16, 2g:2g+2] = u_chunk[128,16].T @ m_chunk[128,2]
    i.e. column 2g = sum of u over the chunk (usums), column 2g+1 = sum of
    n*u = sum of p (sums). 4096 chunks/core, ap_size=2 -> PE nearly free.
  - chunk -> label mapping is known on host (sorted order); label sums are
    prefix-sum differences of chunk sums plus exact host-side corrections
    for the <=50 chunks per batch that straddle a label boundary.
  - per-core DMA: 8 MiB u + 2 MiB m in, 512 KiB chunk sums out
    (vs 24.5 MiB for the one-hot/matmul formulation) -> memory-roofline.
"""

import numpy as np
import ml_dtypes

import concourse.tile as tile
from concourse import bacc, mybir
from concourse.bass_utils import run_bass_kernel_spmd

NUM_LABELS = 51
EPS = 1e-8

N_CORES = 8
B, C, Z, Y, X = 2, 16, 32, 256, 256
NVB = Z * Y * X                     # voxels per batch = 2_097_152
CORES_PER_B = N_CORES // B          # 4
NV_CORE = NVB // CORES_PER_B        # 524_288 voxels per core
P = 128                             # partitions = voxels per chunk
CH = 16                             # channels
G = 512                             # chunks per tile (one PSUM flush)
T = NV_CORE // (P * G)              # 8 tiles per core
CHUNKS_CORE = T * G                 # 4096
CHUNKS_B = CHUNKS_CORE * CORES_PER_B  # 16384 chunks per batch

_cache = {}


def _build_bass():
    nc = bacc.Bacc(
        "TRN2",
        target_bir_lowering=False,
        debug=False,
        enable_asserts=False,
        num_devices=N_CORES,
    )
    u_d = nc.dram_tensor("u", [T, P, G * CH], mybir.dt.float8e4, kind="ExternalInput")
    m_d = nc.dram_tensor("m", [T, P, G * 2], mybir.dt.float8e4, kind="ExternalInput")
    out_d = nc.dram_tensor("out", [T, CH, G * 2], mybir.dt.float32, kind="ExternalOutput")

    with tile.TileContext(nc) as tc:
        with (
            tc.tile_pool(name="upool", bufs=2) as upool,
            tc.tile_pool(name="mpool", bufs=2) as mpool,
            tc.tile_pool(name="rpool", bufs=2) as rpool,
            tc.tile_pool(name="psum", bufs=2, space="PSUM") as psum_pool,
        ):
            for t in range(T):
                ut = upool.tile([P, G * CH], mybir.dt.float8e4)
                mt = mpool.tile([P, G * 2], mybir.dt.float8e4)
                # inputs in-order on the SP HWDGE queue; outputs go on the
                # Activation queue so a flush never stalls the next load
                nc.sync.dma_start(out=ut[:], in_=u_d.ap()[t])
                nc.sync.dma_start(out=mt[:], in_=m_d.ap()[t])

                acc = psum_pool.tile([CH, G * 2], dtype=mybir.dt.float32, space="PSUM")
                for g in range(G):
                    nc.tensor.matmul(
                        out=acc[:, 2 * g : 2 * g + 2],
                        lhsT=ut[:, CH * g : CH * (g + 1)],
                        rhs=mt[:, 2 * g : 2 * g + 2],
                        start=True,
                        stop=True,
                    )
                res = rpool.tile([CH, G * 2], mybir.dt.float32)
                nc.vector.tensor_copy(out=res[:], in_=acc[:])
                nc.scalar.dma_start(out=out_d.ap()[t], in_=res[:])
    nc.compile()
    return nc


def _host_prep(prediction, gt):
    """Sort voxels by label per batch; build per-core device inputs.

    Returns (in_maps, counts, per_batch) where per_batch[b] =
    (u8s [NVB,16] fp8, n16s [NVB] bf16, starts [52] int64) in sorted order.
    """
    pred = np.asarray(prediction, dtype=np.float32)
    ids64 = np.asarray(gt)
    counts = np.bincount(ids64.reshape(-1).astype(np.int64), minlength=NUM_LABELS)

    p = pred.reshape(B, C, -1)
    ids = ids64.reshape(B, -1).astype(np.int32)
    in_maps = [None] * N_CORES
    per_batch = []
    for b in range(B):
        nrm = np.sqrt(np.einsum("cv,cv->v", p[b], p[b]))
        u8 = (p[b] / np.maximum(nrm, 1e-30)[None, :]).astype(ml_dtypes.float8_e4m3fn)
        n8 = nrm.astype(ml_dtypes.float8_e4m3fn)
        counts_b = np.bincount(ids[b], minlength=NUM_LABELS)
        starts = np.zeros(NUM_LABELS + 1, np.int64)
        starts[1:] = np.cumsum(counts_b)
        order = np.argsort(ids[b], kind="stable")
        u8s = np.ascontiguousarray(u8[:, order].T)      # [NVB, 16] fp8
        n8s = np.ascontiguousarray(n8[order])           # [NVB] fp8
        per_batch.append((u8s, n8s, starts))
        for q in range(CORES_PER_B):
            sl = slice(q * NV_CORE, (q + 1) * NV_CORE)
            us = np.ascontiguousarray(
                u8s[sl].reshape(T, G, P, CH).transpose(0, 2, 1, 3)
            ).reshape(T, P, G * CH)
            m = np.empty((T, P, G, 2), ml_dtypes.float8_e4m3fn)
            m[..., 0] = np.asarray(1.0, ml_dtypes.float8_e4m3fn)
            m[..., 1] = n8s[sl].reshape(T, G, P).transpose(0, 2, 1)
            in_maps[b * CORES_PER_B + q] = {"u": us, "m": m.reshape(T, P, G * 2)}
    return in_maps, counts, per_batch


def _host_final(outs, counts, per_batch):
    """outs: per core [T, CH, G*2] fp32 chunk sums. Final reduce in float64."""
    sums = np.zeros((NUM_LABELS, CH), np.float64)
    usums = np.zeros((NUM_LABELS, CH), np.float64)
    for b in range(B):
        u8s, n8s, starts = per_batch[b]
        cs = np.concatenate(
            [
                np.asarray(outs[b * CORES_PER_B + q], np.float64)
                .reshape(T, CH, G, 2)
                .transpose(0, 2, 3, 1)
                .reshape(T * G, 2, CH)
                for q in range(CORES_PER_B)
            ]
        )  # [CHUNKS_B, 2, CH]: [:,0]=usum chunk, [:,1]=psum chunk
        pref = np.zeros((CHUNKS_B + 1, 2, CH), np.float64)
        np.cumsum(cs, axis=0, out=pref[1:])
        for l in range(NUM_LABELS):
            s, e = int(starts[l]), int(starts[l + 1])
            if s == e:
                continue
            lo, hi = -(-s // P), e // P
            if hi > lo:
                usums[l] += pref[hi, 0] - pref[lo, 0]
                sums[l] += pref[hi, 1] - pref[lo, 1]
                head = (s, lo * P)
                tailr = (hi * P, e)
            else:
                head = (s, e)
                tailr = (0, 0)
            for a, z in (head, tailr):
                if z > a:
                    useg = u8s[a:z].astype(np.float64)
                    nseg = n8s[a:z].astype(np.float64)
                    usums[l] += useg.sum(axis=0)
                    sums[l] += (useg * nseg[:, None]).sum(axis=0)

    cnt = counts.astype(np.float64)
    means = sums / np.maximum(cnt, 1.0)[:, None]
    mn = np.linalg.norm(means, axis=1)
    intra_sum = np.einsum("lc,lc->l", usums, means) / np.maximum(mn, 1e-300)
    intra_per_label = intra_sum[1:] / np.maximum(cnt[1:], 1.0)
    intra = intra_per_label.mean()

    cm = means[1:]
    cmn = cm / np.maximum(np.linalg.norm(cm, axis=1, keepdims=True), EPS)
    gram = cmn @ cmn.T
    iu, ju = np.triu_indices(NUM_LABELS - 1, k=1)
    inter = np.clip(gram[iu, ju], 0.0, 1.0).mean()
    return np.float32(inter - intra)


def kernel(prediction, gt):
    in_maps, counts, per_batch = _host_prep(prediction, gt)
    if "nc" not in _cache:
        _cache["nc"] = _build_bass()
    res = run_bass_kernel_spmd(_cache["nc"], in_maps, core_ids=list(range(N_CORES)))
    outs = [r["out"] for r in res.results]
    return _host_final(outs, counts, per_batch)


if __name__ == "__main__":
    rng = np.random.default_rng(0)
    pred = rng.standard_normal((B, C, Z, Y, X), dtype=np.float32)
    gt = rng.integers(0, NUM_LABELS, size=(B, Z, Y, X)).astype(np.int64)
    print("loss:", kernel(pred, gt))


# revision 13
# speedup vs baseline: 3.3195x; 1.0074x over previous
"""Contrastive segment-reduce loss kernel for Trainium2 (8 NeuronCores).

Math (equivalent to the reference):
  counts[l] = #voxels with label l                     (host bincount, exact)
  sums[l,c]  = sum_{v: id_v=l} p[v,c]   = sum n_v * u_v[c]
  usums[l,c] = sum_{v: id_v=l} u_v[c],  u_v = p_v/||p_v||, n_v = ||p_v||
  means = sums / max(counts,1)
  intra_sum[l] = usums[l] . means[l] / ||means[l]||    (== sum of per-voxel cos)
  intra = mean over l=1..50 of intra_sum[l]/max(counts[l],1)
  inter = mean of clip(upper-tri cosine of means[1:],0,1)
  loss = inter - intra
The per-voxel eps clamp max(pn*mn, eps) never binds for this data
(pn ~ chi(16) >= O(1), mn ~ 1e-2), so the factored form is exact.

Device strategy (sort-based, no one-hot, no ids on device):
  - host sorts each batch's voxels by label, ships u = p/||p|| as fp8e4m3
    [128, G, 16] and a moving tensor m[128, G, 2] = [1 | n] bf16
  - device computes PER-CHUNK sums via TensorE only: for 128-voxel chunk g,
      psum[0:16, 2g:2g+2] = u_chunk[128,16].T @ m_chunk[128,2]
    i.e. column 2g = sum of u over the chunk (usums), column 2g+1 = sum of
    n*u = sum of p (sums). 4096 chunks/core, ap_size=2 -> PE nearly free.
  - chunk -> label mapping is known on host (sorted order); label sums are
    prefix-sum differences of chunk sums plus exact host-side corrections
    for the <=50 chunks per batch that straddle a label boundary.
  - per-core DMA: 8 MiB u + 2 MiB m in, 512 KiB chunk sums out
    (vs 24.5 MiB for the one-hot/matmul formulation) -> memory-roofline.
"""

import numpy as np
import ml_dtypes

import concourse.tile as tile
from concourse import bacc, mybir
from concourse.bass_utils import run_bass_kernel_spmd

NUM_LABELS = 51
EPS = 1e-8

N_CORES = 8
B, C, Z, Y, X = 2, 16, 32, 256, 256
NVB = Z * Y * X                     # voxels per batch = 2_097_152
CORES_PER_B = N_CORES // B          # 4
NV_CORE = NVB // CORES_PER_B        # 524_288 voxels per core
P = 128                             # partitions = voxels per chunk
CH = 16                             # channels
G = 512                             # chunks per tile (one PSUM flush)
T = NV_CORE // (P * G)              # 8 tiles per core
CHUNKS_CORE = T * G                 # 4096
CHUNKS_B = CHUNKS_CORE * CORES_PER_B  # 16384 chunks per batch

_cache = {}


def _build_bass():
    nc = bacc.Bacc(
        "TRN2",
        target_bir_lowering=False,
        debug=False,
        enable_asserts=False,
        num_devices=N_CORES,
    )
    u_d = nc.dram_tensor("u", [T, P, G * CH], mybir.dt.float8e4, kind="ExternalInput")
    m_d = nc.dram_tensor("m", [T, P, G * 2], mybir.dt.float8e4, kind="ExternalInput")
    out_d = nc.dram_tensor("out", [T, CH, G * 2], mybir.dt.float32, kind="ExternalOutput")

    with tile.TileContext(nc) as tc:
        with (
            tc.tile_pool(name="upool", bufs=4) as upool,
            tc.tile_pool(name="mpool", bufs=4) as mpool,
            tc.tile_pool(name="rpool", bufs=4) as rpool,
            tc.tile_pool(name="psum", bufs=4, space="PSUM") as psum_pool,
        ):
            for t in range(T):
                ut = upool.tile([P, G * CH], mybir.dt.float8e4)
                mt = mpool.tile([P, G * 2], mybir.dt.float8e4)
                # inputs in-order on the SP HWDGE queue; outputs go on the
                # Activation queue so a flush never stalls the next load
                nc.sync.dma_start(out=ut[:], in_=u_d.ap()[t])
                nc.sync.dma_start(out=mt[:], in_=m_d.ap()[t])

                acc = psum_pool.tile([CH, G * 2], dtype=mybir.dt.float32, space="PSUM")
                for g in range(G):
                    nc.tensor.matmul(
                        out=acc[:, 2 * g : 2 * g + 2],
                        lhsT=ut[:, CH * g : CH * (g + 1)],
                        rhs=mt[:, 2 * g : 2 * g + 2],
                        start=True,
                        stop=True,
                    )
                res = rpool.tile([CH, G * 2], mybir.dt.float32)
                nc.vector.tensor_copy(out=res[:], in_=acc[:])
                nc.scalar.dma_start(out=out_d.ap()[t], in_=res[:])
    nc.compile()
    return nc


def _host_prep(prediction, gt):
    """Sort voxels by label per batch; build per-core device inputs.

    Returns (in_maps, counts, per_batch) where per_batch[b] =
    (u8s [NVB,16] fp8, n16s [NVB] bf16, starts [52] int64) in sorted order.
    """
    pred = np.asarray(prediction, dtype=np.float32)
    ids64 = np.asarray(gt)
    counts = np.bincount(ids64.reshape(-1).astype(np.int64), minlength=NUM_LABELS)

    p = pred.reshape(B, C, -1)
    ids = ids64.reshape(B, -1).astype(np.int32)
    in_maps = [None] * N_CORES
    per_batch = []
    for b in range(B):
        nrm = np.sqrt(np.einsum("cv,cv->v", p[b], p[b]))
        u8 = (p[b] / np.maximum(nrm, 1e-30)[None, :]).astype(ml_dtypes.float8_e4m3fn)
        n8 = nrm.astype(ml_dtypes.float8_e4m3fn)
        counts_b = np.bincount(ids[b], minlength=NUM_LABELS)
        starts = np.zeros(NUM_LABELS + 1, np.int64)
        starts[1:] = np.cumsum(counts_b)
        order = np.argsort(ids[b], kind="stable")
        u8s = np.ascontiguousarray(u8[:, order].T)      # [NVB, 16] fp8
        n8s = np.ascontiguousarray(n8[order])           # [NVB] fp8
        per_batch.append((u8s, n8s, starts))
        for q in range(CORES_PER_B):
            sl = slice(q * NV_CORE, (q + 1) * NV_CORE)
            us = np.ascontiguousarray(
                u8s[sl].reshape(T, G, P, CH).transpose(0, 2, 1, 3)
            ).reshape(T, P, G * CH)
            m = np.empty((T, P, G, 2), ml_dtypes.float8_e4m3fn)
            m[..., 0] = np.asarray(1.0, ml_dtypes.float8_e4m3fn)
            m[..., 1] = n8s[sl].reshape(T, G, P).transpose(0, 2, 1)
            in_maps[b * CORES_PER_B + q] = {"u": us, "m": m.reshape(T, P, G * 2)}
    return in_maps, counts, per_batch


def _host_final(outs, counts, per_batch):
    """outs: per core [T, CH, G*2] fp32 chunk sums. Final reduce in float64."""
    sums = np.zeros((NUM_LABELS, CH), np.float64)
    usums = np.zeros((NUM_LABELS, CH), np.float64)
    for b in range(B):
        u8s, n8s, starts = per_batch[b]
        cs = np.concatenate(
            [
                np.asarray(outs[b * CORES_PER_B + q], np.float64)
                .reshape(T, CH, G, 2)
                .transpose(0, 2, 3, 1)
                .reshape(T * G, 2, CH)
                for q in range(CORES_PER_B)
            ]
        )  # [CHUNKS_B, 2, CH]: [:,0]=usum chunk, [:,1]=psum chunk
        pref = np.zeros((CHUNKS_B + 1, 2, CH), np.float64)
        np.cumsum(cs, axis=0, out=pref[1:])
        for l in range(NUM_LABELS):
            s, e = int(starts[l]), int(starts[l + 1])
            if s == e:
                continue
            lo, hi = -(-s // P), e // P
            if hi > lo:
                usums[l] += pref[hi, 0] - pref[lo, 0]
                sums[l] += pref[hi, 1] - pref[lo, 1]
                head = (s, lo * P)
                tailr = (hi * P, e)
            else:
                head = (s, e)
                tailr = (0, 0)
            for a, z in (head, tailr):
                if z > a:
                    useg = u8s[a:z].astype(np.float64)
                    nseg = n8s[a:z].astype(np.float64)
                    usums[l] += useg.sum(axis=0)
                    sums[l] += (useg * nseg[:, None]).sum(axis=0)

    cnt = counts.astype(np.float64)
    means = sums / np.maximum(cnt, 1.0)[:, None]
    mn = np.linalg.norm(means, axis=1)
    intra_sum = np.einsum("lc,lc->l", usums, means) / np.maximum(mn, 1e-300)
    intra_per_label = intra_sum[1:] / np.maximum(cnt[1:], 1.0)
    intra = intra_per_label.mean()

    cm = means[1:]
    cmn = cm / np.maximum(np.linalg.norm(cm, axis=1, keepdims=True), EPS)
    gram = cmn @ cmn.T
    iu, ju = np.triu_indices(NUM_LABELS - 1, k=1)
    inter = np.clip(gram[iu, ju], 0.0, 1.0).mean()
    return np.float32(inter - intra)


def kernel(prediction, gt):
    in_maps, counts, per_batch = _host_prep(prediction, gt)
    if "nc" not in _cache:
        _cache["nc"] = _build_bass()
    res = run_bass_kernel_spmd(_cache["nc"], in_maps, core_ids=list(range(N_CORES)))
    outs = [r["out"] for r in res.results]
    return _host_final(outs, counts, per_batch)


if __name__ == "__main__":
    rng = np.random.default_rng(0)
    pred = rng.standard_normal((B, C, Z, Y, X), dtype=np.float32)
    gt = rng.integers(0, NUM_LABELS, size=(B, Z, Y, X)).astype(np.int64)
    print("loss:", kernel(pred, gt))
